# revision 1
# baseline (speedup 1.0000x reference)
"""nn_CentroidTeacher Trainium2 Bass kernel.

Contract: kernel(**inputs) takes the FULL unsharded inputs
  segments [8, 1, 256, 256] int, motion [8, 1, 1, 256, 256] f32
and returns the full outputs (offsets_target [8, 2, 256, 256] f32,
thingness [8, 1, 256, 256] f32), matching reference.reference().

Sharding: pure data parallel, one sample per NeuronCore (8 cores).

Device algorithm (per core, pixels laid out [128 partitions, 512 free]):
  1. one-hot oh[p, c, m] = (seg[p,c] == m) for m in 0..63, bf16, built by one
     broadcast tensor_tensor is_equal per slab (DVE).
  2. per-segment sums via 512 accumulating PE matmuls:
     psum[7, 64] += V[:, c, :]^T @ oh[:, c, :], V columns =
     (ones, y_hi, y_lo, x_hi, x_lo, mot_hi, mot_lo) in bf16 hi/lo pairs so the
     fp32 PSUM accumulation is exact to fp32 precision.
  3. stats: PE-transpose psum -> [64 segments, 7]; compute per segment
     is_moving = motion_sum/max(cnt,1) > 0.5, centroid cy, cx;
     tables w1 = mov*(cy+4), w2 = mov*(cx+4)  (0 marks non-moving).
  4. broadcast tables to all partitions via PE transpose + ones matmul.
  5. gather per pixel: g = sum_m oh[p,c,m]*w[m] (exact: one-hot is 0/1),
     via scalar_tensor_tensor product + tensor_reduce over the inner m dim.
  6. combine: th = g1 > 2.5; out_y = th*((g1-4)*127.5 - y*127.5); same for x.
"""
import sys
import contextlib
import functools

sys.path.insert(0, "/opt/trn_rl_repo")

import numpy as np
import ml_dtypes

import concourse.bacc as bacc
import concourse.tile as tile
import concourse.mybir as mybir
from concourse.bass_utils import run_bass_kernel_spmd

dt = mybir.dt
Alu = mybir.AluOpType

N_CORES = 8
B = 8
H = W = 256
M = 64
P = 128
C = 512


def _build_bass(loop_n: int = 1):
    """Build + compile the per-core bass module. loop_n > 1 wraps the compute
    body in a hardware For loop (for timing)."""
    nc = bacc.Bacc("TRN2", target_bir_lowering=False, debug=False,
                   num_devices=N_CORES)
    seg_b = nc.dram_tensor("seg_b", [P, C], dt.bfloat16, kind="ExternalInput").ap()
    iota_b = nc.dram_tensor("iota_b", [P, M], dt.bfloat16, kind="ExternalInput").ap()
    vconst = nc.dram_tensor("vconst", [P, C, 8], dt.bfloat16, kind="ExternalInput").ap()
    ycs_d = nc.dram_tensor("ycs", [P, C], dt.float32, kind="ExternalInput").ap()
    xcs_d = nc.dram_tensor("xcs", [P, C], dt.float32, kind="ExternalInput").ap()
    ones_d = nc.dram_tensor("ones_r", [1, P], dt.float32, kind="ExternalInput").ap()
    ident_d = nc.dram_tensor("ident", [M, M], dt.float32, kind="ExternalInput").ap()

    oy_d = nc.dram_tensor("out_y", [P, C], dt.float32, kind="ExternalOutput").ap()
    ox_d = nc.dram_tensor("out_x", [P, C], dt.float32, kind="ExternalOutput").ap()
    th_d = nc.dram_tensor("out_th", [P, C], dt.float32, kind="ExternalOutput").ap()

    with tile.TileContext(nc) as tc:
        with contextlib.ExitStack() as ctx:
            const = ctx.enter_context(tc.tile_pool(name="const", bufs=1))
            big = ctx.enter_context(tc.tile_pool(name="big", bufs=1))
            work = ctx.enter_context(tc.tile_pool(name="work", bufs=2))
            stat = ctx.enter_context(tc.tile_pool(name="stat", bufs=1))
            pp = ctx.enter_context(tc.tile_pool(name="pp", bufs=1, space="PSUM"))

            seg_bt = const.tile([P, C], dt.bfloat16)
            nc.sync.dma_start(seg_bt[:], seg_b[:])
            iota_t = const.tile([P, M], dt.bfloat16)
            nc.sync.dma_start(iota_t[:], iota_b[:])
            v_t = const.tile([P, C, 8], dt.bfloat16)
            nc.sync.dma_start(v_t[:], vconst[:])
            ycs_t = const.tile([P, C], dt.float32)
            nc.sync.dma_start(ycs_t[:], ycs_d[:])
            xcs_t = const.tile([P, C], dt.float32)
            nc.sync.dma_start(xcs_t[:], xcs_d[:])
            ones_t = const.tile([1, P], dt.float32)
            nc.sync.dma_start(ones_t[:], ones_d[:])
            ident_t = const.tile([M, M], dt.float32)
            nc.sync.dma_start(ident_t[:], ident_d[:])

            loop_cm = tc.For_i(0, loop_n, 1) if loop_n > 1 else contextlib.nullcontext()
            with loop_cm:
                body(nc, tc, const, big, work, stat, pp,
                     seg_bt, iota_t, v_t, ycs_t, xcs_t, ones_t, ident_t,
                     oy_d, ox_d, th_d)
    nc.compile()
    return nc


def body(nc, tc, const, big, work, stat, pp,
         seg_bt, iota_t, v_t, ycs_t, xcs_t, ones_t, ident_t,
         oy_d, ox_d, th_d):
    # ---- one-hot + reduce matmuls
    oh_t = big.tile([P, C, M], dt.bfloat16, tag="oh")
    ps = pp.tile([7, M], dt.float32, tag="stats_ps")
    NSLAB = 8
    CS = C // NSLAB
    for s in range(NSLAB):
        c0 = s * CS
        nc.vector.tensor_tensor(
            out=oh_t[:, c0:c0 + CS, :],
            in0=seg_bt[:, c0:c0 + CS].unsqueeze(2).broadcast_to([P, CS, M]),
            in1=iota_t[:].unsqueeze(1).broadcast_to([P, CS, M]),
            op=Alu.is_equal,
        )
    for c in range(C):
        nc.tensor.matmul(ps[:], lhsT=v_t[:, c, 0:7], rhs=oh_t[:, c, :],
                         start=(c == 0), stop=(c == C - 1))

    # ---- stats on [64 segments, 7]
    ps_t = pp.tile([M, 7], dt.float32, tag="stats_T")
    sb7 = stat.tile([7, M], dt.float32, tag="sb7")
    nc.vector.tensor_copy(sb7[:], ps[:])
    nc.tensor.transpose(ps_t[:], sb7[:], ident_t[0:7, 0:7])
    st = stat.tile([M, 7], dt.float32, tag="st")
    nc.vector.tensor_copy(st[:], ps_t[:])

    px = st[:, 0:1]
    ysum = stat.tile([M, 1], dt.float32, tag="ysum")
    nc.vector.tensor_tensor(out=ysum[:], in0=st[:, 1:2], in1=st[:, 2:3], op=Alu.add)
    xsum = stat.tile([M, 1], dt.float32, tag="xsum")
    nc.vector.tensor_tensor(out=xsum[:], in0=st[:, 3:4], in1=st[:, 4:5], op=Alu.add)
    msum = stat.tile([M, 1], dt.float32, tag="msum")
    nc.vector.tensor_tensor(out=msum[:], in0=st[:, 5:6], in1=st[:, 6:7], op=Alu.add)

    pxc = stat.tile([M, 1], dt.float32, tag="pxc")
    nc.vector.tensor_scalar(out=pxc[:], in0=px, scalar1=1.0, scalar2=None, op0=Alu.max)
    inv = stat.tile([M, 1], dt.float32, tag="inv")
    nc.vector.reciprocal(inv[:], pxc[:])
    gap = stat.tile([M, 1], dt.float32, tag="gap")
    nc.vector.scalar_tensor_tensor(out=gap[:], in0=pxc[:], scalar=-0.5,
                                   in1=msum[:], op0=Alu.mult, op1=Alu.add)
    mov = stat.tile([M, 1], dt.float32, tag="mov")
    nc.vector.tensor_scalar(out=mov[:], in0=gap[:], scalar1=0.0, scalar2=None,
                            op0=Alu.is_gt)
    cy = stat.tile([M, 1], dt.float32, tag="cy")
    nc.vector.tensor_tensor(out=cy[:], in0=ysum[:], in1=inv[:], op=Alu.mult)
    cx = stat.tile([M, 1], dt.float32, tag="cx")
    nc.vector.tensor_tensor(out=cx[:], in0=xsum[:], in1=inv[:], op=Alu.mult)

    w1 = stat.tile([M, 1], dt.float32, tag="w1")
    nc.vector.tensor_scalar(out=w1[:], in0=cy[:], scalar1=4.0, scalar2=None, op0=Alu.add)
    nc.vector.tensor_tensor(out=w1[:], in0=w1[:], in1=mov[:], op=Alu.mult)
    w2 = stat.tile([M, 1], dt.float32, tag="w2")
    nc.vector.tensor_scalar(out=w2[:], in0=cx[:], scalar1=4.0, scalar2=None, op0=Alu.add)
    nc.vector.tensor_tensor(out=w2[:], in0=w2[:], in1=mov[:], op=Alu.mult)

    # ---- broadcast tables to [128, 64]
    def bcast_table(w, name):
        wrow_ps = pp.tile([1, M], dt.float32, tag=f"wrow_{name}")
        nc.tensor.transpose(wrow_ps[:], w[:], ident_t[:])
        wrow = stat.tile([1, M], dt.float32, tag=f"wrowsb_{name}")
        nc.vector.tensor_copy(wrow[:], wrow_ps[:])
        wb_ps = pp.tile([P, M], dt.float32, tag=f"wb_{name}")
        nc.tensor.matmul(wb_ps[:], lhsT=ones_t[:], rhs=wrow[:], start=True, stop=True)
        wb = stat.tile([P, M], dt.float32, tag=f"wbsb_{name}")
        nc.vector.tensor_copy(wb[:], wb_ps[:])
        return wb

    w1b = bcast_table(w1, "w1")
    w2b = bcast_table(w2, "w2")

    # ---- gather + combine
    g1_t = big.tile([P, C], dt.float32, tag="g1")
    g2_t = big.tile([P, C], dt.float32, tag="g2")
    NQ = 4
    CQ = C // NQ
    for g_t, wb in ((g1_t, w1b), (g2_t, w2b)):
        for q in range(NQ):
            c0 = q * CQ
            prod = work.tile([P, CQ, M], dt.float32, tag="prod")
            nc.vector.scalar_tensor_tensor(
                out=prod[:], in0=oh_t[:, c0:c0 + CQ, :], scalar=1.0,
                in1=wb[:].unsqueeze(1).broadcast_to([P, CQ, M]),
                op0=Alu.mult, op1=Alu.mult,
            )
            nc.vector.tensor_reduce(out=g_t[:, c0:c0 + CQ], in_=prod[:],
                                    axis=mybir.AxisListType.X, op=Alu.add)

    th_t = big.tile([P, C], dt.float32, tag="th")
    nc.vector.tensor_scalar(out=th_t[:], in0=g1_t[:], scalar1=2.5, scalar2=None,
                            op0=Alu.is_gt)
    oy_t = big.tile([P, C], dt.float32, tag="oy")
    nc.vector.tensor_scalar(out=oy_t[:], in0=g1_t[:], scalar1=127.5,
                            scalar2=-510.0, op0=Alu.mult, op1=Alu.add)
    nc.vector.tensor_tensor(out=oy_t[:], in0=oy_t[:], in1=ycs_t[:], op=Alu.subtract)
    nc.vector.tensor_tensor(out=oy_t[:], in0=oy_t[:], in1=th_t[:], op=Alu.mult)
    ox_t = big.tile([P, C], dt.float32, tag="ox")
    nc.vector.tensor_scalar(out=ox_t[:], in0=g2_t[:], scalar1=127.5,
                            scalar2=-510.0, op0=Alu.mult, op1=Alu.add)
    nc.vector.tensor_tensor(out=ox_t[:], in0=ox_t[:], in1=xcs_t[:], op=Alu.subtract)
    nc.vector.tensor_tensor(out=ox_t[:], in0=ox_t[:], in1=th_t[:], op=Alu.mult)

    nc.sync.dma_start(oy_d[:], oy_t[:])
    nc.sync.dma_start(ox_d[:], ox_t[:])
    nc.sync.dma_start(th_d[:], th_t[:])


@functools.lru_cache(maxsize=None)
def _consts():
    ys = np.linspace(-1, 1, H, dtype=np.float64)
    xs = np.linspace(-1, 1, W, dtype=np.float64)
    hh = np.arange(H).reshape(P, 2).repeat(W, 1).reshape(P, C)
    ww = np.tile(np.arange(W), (P, 2)).reshape(P, C)
    yv = ys[hh]
    xv = xs[ww]
    vconst = np.zeros((P, C, 8), dtype=ml_dtypes.bfloat16)
    vconst[:, :, 0] = 1.0
    vconst[:, :, 1] = yv.astype(ml_dtypes.bfloat16)
    vconst[:, :, 2] = (yv - vconst[:, :, 1].astype(np.float64)).astype(ml_dtypes.bfloat16)
    vconst[:, :, 3] = xv.astype(ml_dtypes.bfloat16)
    vconst[:, :, 4] = (xv - vconst[:, :, 3].astype(np.float64)).astype(ml_dtypes.bfloat16)
    return {
        "vconst": vconst,
        "iota_b": np.tile(np.arange(M, dtype=np.float32), (P, 1)).astype(ml_dtypes.bfloat16),
        "ycs": (yv * 127.5).astype(np.float32),
        "xcs": (xv * 127.5).astype(np.float32),
        "ones_r": np.ones((1, P), dtype=np.float32),
        "ident": np.eye(M, dtype=np.float32),
        "yv": yv, "xv": xv,
    }


@functools.lru_cache(maxsize=None)
def _get_nc(loop_n: int = 1):
    return _build_bass(loop_n)


def _make_in_maps(segments, motion, vc):
    cst = _consts()
    base = {k: cst[k] for k in ("iota_b", "ycs", "xcs", "ones_r", "ident")}
    in_maps = []
    for b in range(B):
        seg = np.asarray(segments[b, 0]).reshape(P, C)
        mot = np.asarray(motion[b, 0, 0], dtype=np.float32).reshape(P, C)
        vconst = cst["vconst"].copy()
        mh = mot.astype(ml_dtypes.bfloat16)
        vconst[:, :, 5] = mh
        vconst[:, :, 6] = (mot.astype(np.float64) - mh.astype(np.float64)).astype(
            ml_dtypes.bfloat16)
        in_maps.append({
            **base,
            "seg_b": seg.astype(ml_dtypes.bfloat16),
            "vconst": vconst,
        })
    return in_maps


def kernel(segments, motion, _loop_n: int = 1):
    segments = np.asarray(segments)
    motion = np.asarray(motion)
    assert segments.shape == (B, 1, H, W) and motion.shape == (B, 1, 1, H, W)
    nc = _get_nc(_loop_n)
    in_maps = _make_in_maps(segments, motion, None)
    res = run_bass_kernel_spmd(nc, in_maps, core_ids=list(range(N_CORES)))
    offsets = np.empty((B, 2, H, W), dtype=np.float32)
    thing = np.empty((B, 1, H, W), dtype=np.float32)
    for b in range(B):
        offsets[b, 0] = res.results[b]["out_y"].reshape(H, W)
        offsets[b, 1] = res.results[b]["out_x"].reshape(H, W)
        thing[b, 0] = res.results[b]["out_th"].reshape(H, W)
    return offsets, thing
